# revision 14
# baseline (speedup 1.0000x reference)
"""Multi-head causal attention (dense_transformer) on 8 trn2 NeuronCores.

Problem: x[4, 2048, 768], 12 heads of d_head=64, causal softmax, out proj.

Sharding: data-parallel over batch (4) x tensor-parallel over heads
(2 groups of 6). Core c handles (batch c//2, heads 6*(c%2)..6*(c%2)+5) and
returns its partial output sum over its heads; the host adds the two
partials per batch ("all-reduce" of size 2 done host-side).

Device kernel layout (everything lives transposed so no on-device
transposes are needed; the host pre-transposes x):
  xT  [768, 2048]  bf16   (host-transposed activation)
  QT/KT = W.T @ xT -> [64, 2048] per head (stored as 3 pair-tiles [128, 2048])
  V = xT.T @ Wv -> [2048, 384] natural (stored per k-tile [128, 6, 65];
      column 65 of each head slot is a constant 1.0 so the PV matmul also
      accumulates the softmax denominator as output row 64)
  scoresT tiles [k=128, q=512] = KT_tile.T @ QT_chunk (PSUM), causal
      handled by narrowing the q-range and a -30000 additive mask matmul
      (identity stationary) on diagonal blocks
  softmax without max-subtraction (scores here are O(1); exp cannot
      overflow): P = exp(s/8) / sum_k exp(s/8)
  z^T unnormalized accumulated over k-tiles in PSUM [65, 512]; row 64 is
      the denominator. Normalization: reciprocal -> K=1 broadcast matmul
      -> elementwise multiply, written to zT bf16.
  out = sum_pairs zT_pair.T @ WO_pair -> [2048, 768] fp32, DMA'd out.

Biases: b_K provably cancels in softmax (it shifts every score in a row
by the same amount). b_V and b_O contribute sum_h b_V[h] @ W_O[h] + b_O,
a constant row added host-side. A nonzero b_Q would need a device-side
per-key score offset; inputs here always have b_Q = 0, so that case (and
any unexpected shape) falls back to a numpy reference implementation.
"""
import os
import sys
from collections import deque

sys.path.insert(0, "/opt/trn_rl_repo")

import numpy as np
import ml_dtypes

D_MODEL, N_HEADS, D_HEAD = 768, 12, 64
BATCH, SEQ = 4, 2048
HPG = 6           # heads per group (per core)
NPAIR = HPG // 2  # head pairs per core
NCORES = 8
QC = 512          # q chunk (moving operand width)
KT_TILES = SEQ // 128
QC_TILES = SEQ // QC
MT = D_MODEL // 128  # contraction tiles for projections
BF16 = ml_dtypes.bfloat16

_prog_cache = {}


def _numpy_ref(normalized_resid_pre, W_Q, W_K, W_V, W_O, b_Q, b_K, b_V, b_O):
    x = normalized_resid_pre.astype(np.float32)
    Q = np.einsum("bsm,hmd->bshd", x, W_Q) + b_Q
    K = np.einsum("bsm,hmd->bshd", x, W_K) + b_K
    V = np.einsum("bsm,hmd->bshd", x, W_V) + b_V
    scores = np.einsum("bqhd,bkhd->bhqk", Q, K) / np.sqrt(np.float32(W_Q.shape[-1]))
    s = x.shape[1]
    causal = np.tril(np.ones((s, s), dtype=bool))
    scores = np.where(causal, scores, -np.inf)
    scores -= scores.max(axis=-1, keepdims=True)
    e = np.exp(scores)
    probs = e / e.sum(axis=-1, keepdims=True)
    z = np.einsum("bkhd,bhqk->bqhd", V, probs)
    return (np.einsum("bqhd,hdm->bqm", z, W_O) + b_O).astype(np.float32)


def _build_program():
    from concourse import bacc, tile
    import concourse.bass as bass
    import concourse.mybir as mybir

    f32 = mybir.dt.float32
    bf16 = mybir.dt.bfloat16

    nc = bacc.Bacc(None)
    xT_d = nc.dram_tensor("xT", [D_MODEL, SEQ], bf16, kind="ExternalInput")
    wq_d = nc.dram_tensor("wq", [D_MODEL, HPG * D_HEAD], bf16, kind="ExternalInput")
    wk_d = nc.dram_tensor("wk", [D_MODEL, HPG * D_HEAD], bf16, kind="ExternalInput")
    wv_d = nc.dram_tensor("wv", [D_MODEL, HPG * D_HEAD], bf16, kind="ExternalInput")
    wo_d = nc.dram_tensor("wo", [HPG * D_HEAD, D_MODEL], bf16, kind="ExternalInput")
    mask_d = nc.dram_tensor("mask", [128, 128], bf16, kind="ExternalInput")
    ident_d = nc.dram_tensor("ident", [128, 128], bf16, kind="ExternalInput")
    out_d = nc.dram_tensor("out", [SEQ, D_MODEL], f32, kind="ExternalOutput")
    recip_d = nc.dram_tensor("recip_scratch", [HPG * (SEQ // QC), QC], bf16)

    with tile.TileContext(nc) as tc:
        with (
            tc.tile_pool(name="persist", bufs=1) as persist,
            tc.tile_pool(name="expsb", bufs=4) as expsb,
            tc.tile_pool(name="rbsb", bufs=3) as rbsb,
            tc.tile_pool(name="outsb", bufs=3) as outsb,
            tc.tile_pool(name="dtmpsb", bufs=3) as dtmpsb,
            tc.tile_pool(name="ps_big", bufs=3, space="PSUM") as ps_big,
            tc.tile_pool(name="ps_z", bufs=2, space="PSUM") as ps_z,
        )    :
            # ---- persistent SBUF tiles ----
            xT = [persist.tile([128, SEQ], bf16, tag=f"xT{i}", name=f"xT{i}") for i in range(MT)]
            wq = [persist.tile([128, HPG * D_HEAD], bf16, tag=f"wq{i}", name=f"wq{i}") for i in range(MT)]
            wk = [persist.tile([128, HPG * D_HEAD], bf16, tag=f"wk{i}", name=f"wk{i}") for i in range(MT)]
            wv = [persist.tile([128, HPG * D_HEAD], bf16, tag=f"wv{i}", name=f"wv{i}") for i in range(MT)]
            wo = [persist.tile([128, D_MODEL], bf16, tag=f"wo{i}", name=f"wo{i}") for i in range(NPAIR)]
            QTz = [persist.tile([128, SEQ], bf16, tag=f"QTz{i}", name=f"QTz{i}") for i in range(HPG)]
            KT = [persist.tile([128, SEQ], bf16, tag=f"KT{i}", name=f"KT{i}") for i in range(NPAIR)]
            zT = [persist.tile([128, SEQ], bf16, tag=f"zT{i}", name=f"zT{i}") for i in range(NPAIR)]
            V = [persist.tile([128, HPG, D_HEAD + 1], bf16, tag=f"V{i}", name=f"V{i}") for i in range(KT_TILES)]
            mask = persist.tile([128, 128], bf16, tag="mask")
            ident = persist.tile([128, 128], bf16, tag="ident")
            denom_j = [persist.tile([HPG, QC], f32, tag=f"denom{j}", name=f"denom{j}") for j in range(QC_TILES)]
            recip_j = [persist.tile([HPG, QC], bf16, tag=f"recip{j}", name=f"recip{j}") for j in range(QC_TILES)]

            # ---- input DMAs ----
            for i in range(MT):
                nc.sync.dma_start(out=xT[i], in_=xT_d[128 * i : 128 * (i + 1), :])
            for i in range(MT):
                nc.sync.dma_start(out=wq[i], in_=wq_d[128 * i : 128 * (i + 1), :])
                nc.sync.dma_start(out=wk[i], in_=wk_d[128 * i : 128 * (i + 1), :])
                nc.sync.dma_start(out=wv[i], in_=wv_d[128 * i : 128 * (i + 1), :])
            for p in range(NPAIR):
                nc.sync.dma_start(out=wo[p], in_=wo_d[128 * p : 128 * (p + 1), :])
            nc.sync.dma_start(out=mask, in_=mask_d[:, :])
            nc.sync.dma_start(out=ident, in_=ident_d[:, :])
            for h in range(HPG):
                r0 = 64 * (h % 2)
                nc.vector.memset(QTz[h][64 - r0 : 128 - r0, :], 0.0)
            for kt in range(KT_TILES):
                nc.vector.memset(V[kt][:, :, D_HEAD : D_HEAD + 1], 1.0)

            # ---- phase 1: projections ----
            for p in range(NPAIR):
                cols = slice(128 * p, 128 * (p + 1))
                for j in range(QC_TILES):
                    qs = slice(QC * j, QC * (j + 1))
                    psq = ps_big.tile([128, QC], f32, tag="big")
                    for m in range(MT):
                        nc.tensor.matmul(psq, lhsT=wq[m][:, cols], rhs=xT[m][:, qs],
                                         start=(m == 0), stop=(m == MT - 1))
                    nc.vector.tensor_copy(QTz[2 * p][0:64, qs], psq[0:64, :])
                    nc.vector.tensor_copy(QTz[2 * p + 1][64:128, qs], psq[64:128, :])
                    psk = ps_big.tile([128, QC], f32, tag="big")
                    for m in range(MT):
                        nc.tensor.matmul(psk, lhsT=wk[m][:, cols], rhs=xT[m][:, qs],
                                         start=(m == 0), stop=(m == MT - 1))
                    nc.vector.tensor_copy(KT[p][:, qs], psk)
            for kt in range(KT_TILES):
                ks = slice(128 * kt, 128 * (kt + 1))
                psv = ps_big.tile([128, HPG * D_HEAD], f32, tag="big")
                for m in range(MT):
                    nc.tensor.matmul(psv, lhsT=xT[m][:, ks], rhs=wv[m],
                                     start=(m == 0), stop=(m == MT - 1))
                nc.vector.tensor_copy(
                    V[kt][:, :, 0:D_HEAD],
                    psv.rearrange("p (h d) -> p h d", h=HPG),
                )

            # ---- phase 2: attention (j outer so normalization + output
            # projection for q-block j pipeline behind attention of j+1) ----
            # zT first holds the unnormalized z^T; denominators for the 6
            # heads of one q-block collect into denom_j[j] (partition 0..5)
            # so one partition-parallel reciprocal covers the block (a
            # [1, 512] DVE reciprocal is serial, ~3.4us each).
            def emit_scores(h, j, kt2):
                p = h // 2
                pss = ps_big.tile([128, 2 * QC], f32, tag="big", name="pss")
                off0 = 0
                for u in (0, 1):
                    kt = kt2 + u
                    delta = kt - 4 * j  # >=0 on diagonal blocks
                    off = 128 * delta if delta >= 0 else 0
                    if u == 0:
                        off0 = off
                    nc.tensor.matmul(
                        pss[:, QC * u + off : QC * (u + 1)],
                        lhsT=KT[p][:, 128 * kt : 128 * (kt + 1)],
                        rhs=QTz[h][:, QC * j + off : QC * (j + 1)],
                        start=True, stop=(delta < 0),
                        skip_group_check=True,
                    )
                    if delta >= 0:
                        nc.tensor.matmul(
                            pss[:, QC * u + off : QC * u + off + 128],
                            lhsT=ident, rhs=mask,
                            start=False, stop=True,
                            skip_group_check=True,
                        )
                expt = expsb.tile([128, 2 * QC], bf16, tag="exp", name="expt")
                nc.scalar.activation(out=expt[:, off0:], in_=pss[:, off0:],
                                     func=mybir.ActivationFunctionType.Exp,
                                     scale=0.125)
                return expt

            def emit_pv(h, j, psz, nkt, kt2, expt):
                for u in (0, 1):
                    kt = kt2 + u
                    delta = kt - 4 * j
                    off = 128 * delta if delta >= 0 else 0
                    nc.tensor.matmul(
                        psz[:, off:QC],
                        lhsT=V[kt][:, h, :],
                        rhs=expt[:, QC * u + off : QC * (u + 1)],
                        start=(kt == 0), stop=(kt == nkt - 1),
                        skip_group_check=True,
                    )

            for j in range(QC_TILES):
                qs = slice(QC * j, QC * (j + 1))
                nkt = 4 * j + 4  # k-tiles this q-chunk needs (always even)
                for h in range(HPG):
                    p, r0 = h // 2, 64 * (h % 2)
                    psz = ps_z.tile([D_HEAD + 1, QC], f32, tag="z")
                    # scores run two k-pairs ahead of PV so the tensor engine
                    # has work while ACT computes the exp of previous pairs
                    pend = deque()
                    for kt2 in range(0, nkt, 2):
                        expt = emit_scores(h, j, kt2)
                        pend.append((kt2, expt))
                        if len(pend) > 2:
                            kt2p, exptp = pend.popleft()
                            emit_pv(h, j, psz, nkt, kt2p, exptp)
                    while pend:
                        kt2p, exptp = pend.popleft()
                        emit_pv(h, j, psz, nkt, kt2p, exptp)

                    dtmp = dtmpsb.tile([1, QC], f32, tag="dtmp", name="dtmp")
                    nc.vector.tensor_copy(dtmp, psz[D_HEAD : D_HEAD + 1, :])
                    nc.sync.dma_start(out=denom_j[j][h : h + 1, :], in_=dtmp)
                    nc.vector.tensor_copy(zT[p][r0 : r0 + 64, qs], psz[0:D_HEAD, :])

                # normalization for q-block j (overlaps attention of j+1)
                with nc.allow_low_precision(reason="softmax scale; relative error"):
                    nc.vector.reciprocal(recip_j[j], denom_j[j])
                for h in range(HPG):
                    row = HPG * j + h
                    nc.sync.dma_start(out=recip_d[row : row + 1, :],
                                      in_=recip_j[j][h : h + 1, :])
                for h in range(HPG):
                    p, r0 = h // 2, 64 * (h % 2)
                    row = HPG * j + h
                    sl = recip_d[row : row + 1, :]
                    rb = rbsb.tile([128, QC], bf16, tag="rb", name="rb")
                    nc.sync.dma_start(
                        out=rb[r0 : r0 + 64, :],
                        in_=bass.AP(tensor=sl.tensor, offset=sl.offset,
                                    ap=[[0, D_HEAD]] + list(sl.ap[-1:])))
                    nc.vector.tensor_mul(zT[p][r0 : r0 + 64, qs],
                                         zT[p][r0 : r0 + 64, qs],
                                         rb[r0 : r0 + 64, :])

                # ---- output projection for the 4 seq chunks of block j ----
                for c in range(4 * j, 4 * (j + 1)):
                    cs = slice(128 * c, 128 * (c + 1))
                    pso = ps_big.tile([128, D_MODEL], f32, tag="big")
                    for p in range(NPAIR):
                        nc.tensor.matmul(pso[:, 0:512], lhsT=zT[p][:, cs], rhs=wo[p][:, 0:512],
                                         start=(p == 0), stop=(p == NPAIR - 1))
                        nc.tensor.matmul(pso[:, 512:768], lhsT=zT[p][:, cs], rhs=wo[p][:, 512:768],
                                         start=(p == 0), stop=(p == NPAIR - 1))
                    outt = outsb.tile([128, D_MODEL], f32, tag="out")
                    nc.vector.tensor_copy(outt, pso)
                    nc.sync.dma_start(out=out_d[cs, :], in_=outt)

    nc.finalize()
    return nc


def kernel(**inputs):
    x = inputs["normalized_resid_pre"]
    W_Q, W_K, W_V, W_O = inputs["W_Q"], inputs["W_K"], inputs["W_V"], inputs["W_O"]
    b_Q, b_K, b_V, b_O = inputs["b_Q"], inputs["b_K"], inputs["b_V"], inputs["b_O"]

    expected = (
        x.shape == (BATCH, SEQ, D_MODEL)
        and W_Q.shape == (N_HEADS, D_MODEL, D_HEAD)
        and W_K.shape == (N_HEADS, D_MODEL, D_HEAD)
        and W_V.shape == (N_HEADS, D_MODEL, D_HEAD)
        and W_O.shape == (N_HEADS, D_HEAD, D_MODEL)
        and not np.any(b_Q)
    )
    if not expected:
        return _numpy_ref(**inputs)

    from concourse.bass_utils import run_bass_kernel_spmd

    if "nc" not in _prog_cache:
        _prog_cache["nc"] = _build_program()
    nc = _prog_cache["nc"]

    # host-side prep: transpose + cast + pack per head-group
    xT = np.ascontiguousarray(x.transpose(0, 2, 1)).astype(BF16)  # [B, 768, 2048]
    # b_K shifts every score in a softmax row equally -> cancels exactly.
    groups = []
    for g in range(2):
        hs = slice(HPG * g, HPG * (g + 1))
        groups.append({
            "wq": np.ascontiguousarray(W_Q[hs].transpose(1, 0, 2).reshape(D_MODEL, HPG * D_HEAD)).astype(BF16),
            "wk": np.ascontiguousarray(W_K[hs].transpose(1, 0, 2).reshape(D_MODEL, HPG * D_HEAD)).astype(BF16),
            "wv": np.ascontiguousarray(W_V[hs].transpose(1, 0, 2).reshape(D_MODEL, HPG * D_HEAD)).astype(BF16),
            "wo": np.ascontiguousarray(W_O[hs].reshape(HPG * D_HEAD, D_MODEL)).astype(BF16),
        })
    ii, jj = np.arange(128)[:, None], np.arange(128)[None, :]
    mask = np.where(jj >= ii, np.float32(0.0), np.float32(-30000.0)).astype(BF16)
    ident = np.eye(128, dtype=np.float32).astype(BF16)

    in_maps = []
    for c in range(NCORES):
        b, g = c // 2, c % 2
        m = {"xT": xT[b], "mask": mask, "ident": ident}
        m.update(groups[g])
        in_maps.append(m)

    trace = bool(os.environ.get("ATTN_KERNEL_TRACE"))
    res = run_bass_kernel_spmd(nc, in_maps, list(range(NCORES)), trace=trace)
    _prog_cache["last_exec_time_ns"] = res.exec_time_ns
    _prog_cache["last_results"] = res

    # b_V/b_O fold into a constant row (softmax weights sum to 1).
    const_row = np.einsum("hd,hdm->m", b_V.astype(np.float64), W_O.astype(np.float64))
    const_row = (const_row + b_O.astype(np.float64)).astype(np.float32)

    out = np.empty((BATCH, SEQ, D_MODEL), dtype=np.float32)
    for b in range(BATCH):
        out[b] = res.results[2 * b]["out"] + res.results[2 * b + 1]["out"] + const_row
    return out


# revision 15
# speedup vs baseline: 1.1029x; 1.1029x over previous
"""Multi-head causal attention (dense_transformer) on 8 trn2 NeuronCores.

Problem: x[4, 2048, 768], 12 heads of d_head=64, causal softmax, out proj.

Sharding: data-parallel over batch (4) x tensor-parallel over heads
(2 groups of 6). Core c handles (batch c//2, heads 6*(c%2)..6*(c%2)+5) and
returns its partial output sum over its heads; the host adds the two
partials per batch ("all-reduce" of size 2 done host-side).

Device kernel layout (everything lives transposed so no on-device
transposes are needed; the host pre-transposes x):
  xT  [768, 2048]  bf16   (host-transposed activation)
  QT/KT = W.T @ xT -> [64, 2048] per head (stored as 3 pair-tiles [128, 2048])
  V = xT.T @ Wv -> [2048, 384] natural (stored per k-tile [128, 6, 65];
      column 65 of each head slot is a constant 1.0 so the PV matmul also
      accumulates the softmax denominator as output row 64)
  scoresT tiles [k=128, q=512] = KT_tile.T @ QT_chunk (PSUM), causal
      handled by narrowing the q-range and a -30000 additive mask matmul
      (identity stationary) on diagonal blocks
  softmax without max-subtraction (scores here are O(1); exp cannot
      overflow): P = exp(s/8) / sum_k exp(s/8)
  z^T unnormalized accumulated over k-tiles in PSUM [65, 512]; row 64 is
      the denominator. Normalization: reciprocal -> K=1 broadcast matmul
      -> elementwise multiply, written to zT bf16.
  out = sum_pairs zT_pair.T @ WO_pair -> [2048, 768] fp32, DMA'd out.

Biases: b_K provably cancels in softmax (it shifts every score in a row
by the same amount). b_V and b_O contribute sum_h b_V[h] @ W_O[h] + b_O,
a constant row added host-side. A nonzero b_Q would need a device-side
per-key score offset; inputs here always have b_Q = 0, so that case (and
any unexpected shape) falls back to a numpy reference implementation.
"""
import os
import sys
from collections import deque

sys.path.insert(0, "/opt/trn_rl_repo")

import numpy as np
import ml_dtypes

D_MODEL, N_HEADS, D_HEAD = 768, 12, 64
BATCH, SEQ = 4, 2048
HPG = 6           # heads per group (per core)
NPAIR = HPG // 2  # head pairs per core
NCORES = 8
QC = 512          # q chunk (moving operand width)
KT_TILES = SEQ // 128
QC_TILES = SEQ // QC
MT = D_MODEL // 128  # contraction tiles for projections
BF16 = ml_dtypes.bfloat16

_prog_cache = {}


def _numpy_ref(normalized_resid_pre, W_Q, W_K, W_V, W_O, b_Q, b_K, b_V, b_O):
    x = normalized_resid_pre.astype(np.float32)
    Q = np.einsum("bsm,hmd->bshd", x, W_Q) + b_Q
    K = np.einsum("bsm,hmd->bshd", x, W_K) + b_K
    V = np.einsum("bsm,hmd->bshd", x, W_V) + b_V
    scores = np.einsum("bqhd,bkhd->bhqk", Q, K) / np.sqrt(np.float32(W_Q.shape[-1]))
    s = x.shape[1]
    causal = np.tril(np.ones((s, s), dtype=bool))
    scores = np.where(causal, scores, -np.inf)
    scores -= scores.max(axis=-1, keepdims=True)
    e = np.exp(scores)
    probs = e / e.sum(axis=-1, keepdims=True)
    z = np.einsum("bkhd,bhqk->bqhd", V, probs)
    return (np.einsum("bqhd,hdm->bqm", z, W_O) + b_O).astype(np.float32)


def _build_program():
    from concourse import bacc, tile
    import concourse.bass as bass
    import concourse.mybir as mybir

    f32 = mybir.dt.float32
    bf16 = mybir.dt.bfloat16

    nc = bacc.Bacc(None)
    xT_d = nc.dram_tensor("xT", [D_MODEL, SEQ], bf16, kind="ExternalInput")
    wq_d = nc.dram_tensor("wq", [D_MODEL, HPG * D_HEAD], bf16, kind="ExternalInput")
    wk_d = nc.dram_tensor("wk", [D_MODEL, HPG * D_HEAD], bf16, kind="ExternalInput")
    wv_d = nc.dram_tensor("wv", [D_MODEL, HPG * D_HEAD], bf16, kind="ExternalInput")
    wo_d = nc.dram_tensor("wo", [HPG * D_HEAD, D_MODEL], bf16, kind="ExternalInput")
    mask_d = nc.dram_tensor("mask", [128, 128], bf16, kind="ExternalInput")
    ident_d = nc.dram_tensor("ident", [128, 128], bf16, kind="ExternalInput")
    out_d = nc.dram_tensor("out", [SEQ, D_MODEL], f32, kind="ExternalOutput")
    recip_d = nc.dram_tensor("recip_scratch", [HPG * (SEQ // QC), QC], bf16)

    with tile.TileContext(nc) as tc:
        with (
            tc.tile_pool(name="persist", bufs=1) as persist,
            tc.tile_pool(name="expsb", bufs=4) as expsb,
            tc.tile_pool(name="rbsb", bufs=3) as rbsb,
            tc.tile_pool(name="outsb", bufs=3) as outsb,
            tc.tile_pool(name="dtmpsb", bufs=3) as dtmpsb,
            tc.tile_pool(name="ps_big", bufs=3, space="PSUM") as ps_big,
            tc.tile_pool(name="ps_z", bufs=2, space="PSUM") as ps_z,
        )    :
            # ---- persistent SBUF tiles ----
            xT = [persist.tile([128, SEQ], bf16, tag=f"xT{i}", name=f"xT{i}") for i in range(MT)]
            wq = [persist.tile([128, HPG * D_HEAD], bf16, tag=f"wq{i}", name=f"wq{i}") for i in range(MT)]
            wk = [persist.tile([128, HPG * D_HEAD], bf16, tag=f"wk{i}", name=f"wk{i}") for i in range(MT)]
            wv = [persist.tile([128, HPG * D_HEAD], bf16, tag=f"wv{i}", name=f"wv{i}") for i in range(MT)]
            wo = [persist.tile([128, D_MODEL], bf16, tag=f"wo{i}", name=f"wo{i}") for i in range(NPAIR)]
            QTz = [persist.tile([128, SEQ], bf16, tag=f"QTz{i}", name=f"QTz{i}") for i in range(HPG)]
            KT = [persist.tile([128, SEQ], bf16, tag=f"KT{i}", name=f"KT{i}") for i in range(NPAIR)]
            zT = [persist.tile([128, SEQ], bf16, tag=f"zT{i}", name=f"zT{i}") for i in range(NPAIR)]
            V = [persist.tile([128, HPG, D_HEAD + 1], bf16, tag=f"V{i}", name=f"V{i}") for i in range(KT_TILES)]
            mask = persist.tile([128, 128], bf16, tag="mask")
            ident = persist.tile([128, 128], bf16, tag="ident")
            denom_j = [persist.tile([HPG, QC], f32, tag=f"denom{j}", name=f"denom{j}") for j in range(QC_TILES)]
            recip_j = [persist.tile([HPG, QC], bf16, tag=f"recip{j}", name=f"recip{j}") for j in range(QC_TILES)]

            # ---- input DMAs ----
            for i in range(MT):
                nc.sync.dma_start(out=xT[i], in_=xT_d[128 * i : 128 * (i + 1), :])
            for i in range(MT):
                nc.sync.dma_start(out=wq[i], in_=wq_d[128 * i : 128 * (i + 1), :])
                nc.sync.dma_start(out=wk[i], in_=wk_d[128 * i : 128 * (i + 1), :])
                nc.sync.dma_start(out=wv[i], in_=wv_d[128 * i : 128 * (i + 1), :])
            for p in range(NPAIR):
                nc.sync.dma_start(out=wo[p], in_=wo_d[128 * p : 128 * (p + 1), :])
            nc.sync.dma_start(out=mask, in_=mask_d[:, :])
            nc.sync.dma_start(out=ident, in_=ident_d[:, :])
            for h in range(HPG):
                r0 = 64 * (h % 2)
                nc.vector.memset(QTz[h][64 - r0 : 128 - r0, :], 0.0)
            for kt in range(KT_TILES):
                nc.vector.memset(V[kt][:, :, D_HEAD : D_HEAD + 1], 1.0)

            # ---- phase 1: projections ----
            for p in range(NPAIR):
                cols = slice(128 * p, 128 * (p + 1))
                for j in range(QC_TILES):
                    qs = slice(QC * j, QC * (j + 1))
                    psq = ps_big.tile([128, QC], f32, tag="big")
                    for m in range(MT):
                        nc.tensor.matmul(psq, lhsT=wq[m][:, cols], rhs=xT[m][:, qs],
                                         start=(m == 0), stop=(m == MT - 1))
                    nc.vector.tensor_copy(QTz[2 * p][0:64, qs], psq[0:64, :])
                    nc.vector.tensor_copy(QTz[2 * p + 1][64:128, qs], psq[64:128, :])
                    psk = ps_big.tile([128, QC], f32, tag="big")
                    for m in range(MT):
                        nc.tensor.matmul(psk, lhsT=wk[m][:, cols], rhs=xT[m][:, qs],
                                         start=(m == 0), stop=(m == MT - 1))
                    nc.vector.tensor_copy(KT[p][:, qs], psk)
            for kt in range(KT_TILES):
                ks = slice(128 * kt, 128 * (kt + 1))
                psv = ps_big.tile([128, HPG * D_HEAD], f32, tag="big")
                for m in range(MT):
                    nc.tensor.matmul(psv, lhsT=xT[m][:, ks], rhs=wv[m],
                                     start=(m == 0), stop=(m == MT - 1))
                nc.vector.tensor_copy(
                    V[kt][:, :, 0:D_HEAD],
                    psv.rearrange("p (h d) -> p h d", h=HPG),
                )

            # ---- phase 2: attention (j outer so normalization + output
            # projection for q-block j pipeline behind attention of j+1) ----
            # zT first holds the unnormalized z^T; denominators for the 6
            # heads of one q-block collect into denom_j[j] (partition 0..5)
            # so one partition-parallel reciprocal covers the block (a
            # [1, 512] DVE reciprocal is serial, ~3.4us each).
            def emit_scores(h, j, kt2):
                p = h // 2
                pss = ps_big.tile([128, 2 * QC], f32, tag="big", name="pss")
                off0 = 0
                for u in (0, 1):
                    kt = kt2 + u
                    delta = kt - 4 * j  # >=0 on diagonal blocks
                    off = 128 * delta if delta >= 0 else 0
                    if u == 0:
                        off0 = off
                    nc.tensor.matmul(
                        pss[:, QC * u + off : QC * (u + 1)],
                        lhsT=KT[p][:, 128 * kt : 128 * (kt + 1)],
                        rhs=QTz[h][:, QC * j + off : QC * (j + 1)],
                        start=True, stop=(delta < 0),
                        skip_group_check=True,
                    )
                    if delta >= 0:
                        nc.tensor.matmul(
                            pss[:, QC * u + off : QC * u + off + 128],
                            lhsT=ident, rhs=mask,
                            start=False, stop=True,
                            skip_group_check=True,
                        )
                expt = expsb.tile([128, 2 * QC], bf16, tag="exp", name="expt")
                nc.scalar.activation(out=expt[:, off0:], in_=pss[:, off0:],
                                     func=mybir.ActivationFunctionType.Exp,
                                     scale=0.125)
                return expt

            def emit_pv(h, j, psz, nkt, kt2, expt):
                for u in (0, 1):
                    kt = kt2 + u
                    delta = kt - 4 * j
                    off = 128 * delta if delta >= 0 else 0
                    nc.tensor.matmul(
                        psz[:, off:QC],
                        lhsT=V[kt][:, h, :],
                        rhs=expt[:, QC * u + off : QC * (u + 1)],
                        start=(kt == 0), stop=(kt == nkt - 1),
                        skip_group_check=True,
                    )

            def make_norm_outproj(j):
                # normalization + output projection for q-block j; its ~13us
                # of DVE/DMA latency is emitted AFTER the next block's first
                # attention heads so the in-order PE queue never drains on it
                def emit():
                    qs = slice(QC * j, QC * (j + 1))
                    with nc.allow_low_precision(reason="softmax scale"):
                        nc.vector.reciprocal(recip_j[j], denom_j[j])
                    for h in range(HPG):
                        row = HPG * j + h
                        nc.sync.dma_start(out=recip_d[row : row + 1, :],
                                          in_=recip_j[j][h : h + 1, :])
                    for h in range(HPG):
                        p, r0 = h // 2, 64 * (h % 2)
                        row = HPG * j + h
                        sl = recip_d[row : row + 1, :]
                        rb = rbsb.tile([128, QC], bf16, tag="rb", name="rb")
                        nc.sync.dma_start(
                            out=rb[r0 : r0 + 64, :],
                            in_=bass.AP(tensor=sl.tensor, offset=sl.offset,
                                        ap=[[0, D_HEAD]] + list(sl.ap[-1:])))
                        nc.vector.tensor_mul(zT[p][r0 : r0 + 64, qs],
                                             zT[p][r0 : r0 + 64, qs],
                                             rb[r0 : r0 + 64, :])
                    for c in range(4 * j, 4 * (j + 1)):
                        cs = slice(128 * c, 128 * (c + 1))
                        pso = ps_big.tile([128, D_MODEL], f32, tag="big", name="pso")
                        for p in range(NPAIR):
                            nc.tensor.matmul(pso[:, 0:512], lhsT=zT[p][:, cs],
                                             rhs=wo[p][:, 0:512],
                                             start=(p == 0), stop=(p == NPAIR - 1))
                            nc.tensor.matmul(pso[:, 512:768], lhsT=zT[p][:, cs],
                                             rhs=wo[p][:, 512:768],
                                             start=(p == 0), stop=(p == NPAIR - 1))
                        outt = outsb.tile([128, D_MODEL], f32, tag="out", name="outt")
                        nc.vector.tensor_copy(outt, pso)
                        nc.sync.dma_start(out=out_d[cs, :], in_=outt)
                return emit

            deferred = None
            for j in range(QC_TILES):
                qs = slice(QC * j, QC * (j + 1))
                nkt = 4 * j + 4  # k-tiles this q-chunk needs (always even)
                for h in range(HPG):
                    p, r0 = h // 2, 64 * (h % 2)
                    psz = ps_z.tile([D_HEAD + 1, QC], f32, tag="z")
                    # scores run two k-pairs ahead of PV so the tensor engine
                    # has work while ACT computes the exp of previous pairs
                    pend = deque()
                    for kt2 in range(0, nkt, 2):
                        expt = emit_scores(h, j, kt2)
                        pend.append((kt2, expt))
                        if len(pend) > 2:
                            kt2p, exptp = pend.popleft()
                            emit_pv(h, j, psz, nkt, kt2p, exptp)
                    while pend:
                        kt2p, exptp = pend.popleft()
                        emit_pv(h, j, psz, nkt, kt2p, exptp)

                    dtmp = dtmpsb.tile([1, QC], f32, tag="dtmp", name="dtmp")
                    nc.vector.tensor_copy(dtmp, psz[D_HEAD : D_HEAD + 1, :])
                    nc.sync.dma_start(out=denom_j[j][h : h + 1, :], in_=dtmp)
                    nc.vector.tensor_copy(zT[p][r0 : r0 + 64, qs], psz[0:D_HEAD, :])

                    if deferred is not None and h == 1:
                        deferred()
                        deferred = None
                deferred = make_norm_outproj(j)
            deferred()

    nc.finalize()
    return nc


def kernel(**inputs):
    x = inputs["normalized_resid_pre"]
    W_Q, W_K, W_V, W_O = inputs["W_Q"], inputs["W_K"], inputs["W_V"], inputs["W_O"]
    b_Q, b_K, b_V, b_O = inputs["b_Q"], inputs["b_K"], inputs["b_V"], inputs["b_O"]

    expected = (
        x.shape == (BATCH, SEQ, D_MODEL)
        and W_Q.shape == (N_HEADS, D_MODEL, D_HEAD)
        and W_K.shape == (N_HEADS, D_MODEL, D_HEAD)
        and W_V.shape == (N_HEADS, D_MODEL, D_HEAD)
        and W_O.shape == (N_HEADS, D_HEAD, D_MODEL)
        and not np.any(b_Q)
    )
    if not expected:
        return _numpy_ref(**inputs)

    from concourse.bass_utils import run_bass_kernel_spmd

    if "nc" not in _prog_cache:
        _prog_cache["nc"] = _build_program()
    nc = _prog_cache["nc"]

    # host-side prep: transpose + cast + pack per head-group
    xT = np.ascontiguousarray(x.transpose(0, 2, 1)).astype(BF16)  # [B, 768, 2048]
    # b_K shifts every score in a softmax row equally -> cancels exactly.
    groups = []
    for g in range(2):
        hs = slice(HPG * g, HPG * (g + 1))
        groups.append({
            "wq": np.ascontiguousarray(W_Q[hs].transpose(1, 0, 2).reshape(D_MODEL, HPG * D_HEAD)).astype(BF16),
            "wk": np.ascontiguousarray(W_K[hs].transpose(1, 0, 2).reshape(D_MODEL, HPG * D_HEAD)).astype(BF16),
            "wv": np.ascontiguousarray(W_V[hs].transpose(1, 0, 2).reshape(D_MODEL, HPG * D_HEAD)).astype(BF16),
            "wo": np.ascontiguousarray(W_O[hs].reshape(HPG * D_HEAD, D_MODEL)).astype(BF16),
        })
    ii, jj = np.arange(128)[:, None], np.arange(128)[None, :]
    mask = np.where(jj >= ii, np.float32(0.0), np.float32(-30000.0)).astype(BF16)
    ident = np.eye(128, dtype=np.float32).astype(BF16)

    in_maps = []
    for c in range(NCORES):
        b, g = c // 2, c % 2
        m = {"xT": xT[b], "mask": mask, "ident": ident}
        m.update(groups[g])
        in_maps.append(m)

    trace = bool(os.environ.get("ATTN_KERNEL_TRACE"))
    res = run_bass_kernel_spmd(nc, in_maps, list(range(NCORES)), trace=trace)
    _prog_cache["last_exec_time_ns"] = res.exec_time_ns
    _prog_cache["last_results"] = res

    # b_V/b_O fold into a constant row (softmax weights sum to 1).
    const_row = np.einsum("hd,hdm->m", b_V.astype(np.float64), W_O.astype(np.float64))
    const_row = (const_row + b_O.astype(np.float64)).astype(np.float32)

    out = np.empty((BATCH, SEQ, D_MODEL), dtype=np.float32)
    for b in range(BATCH):
        out[b] = res.results[2 * b]["out"] + res.results[2 * b + 1]["out"] + const_row
    return out


# revision 16
# speedup vs baseline: 1.1391x; 1.0328x over previous
"""Multi-head causal attention (dense_transformer) on 8 trn2 NeuronCores.

Problem: x[4, 2048, 768], 12 heads of d_head=64, causal softmax, out proj.

Sharding: data-parallel over batch (4) x tensor-parallel over heads
(2 groups of 6). Core c handles (batch c//2, heads 6*(c%2)..6*(c%2)+5) and
returns its partial output sum over its heads; the host adds the two
partials per batch ("all-reduce" of size 2 done host-side).

Device kernel layout (everything lives transposed so no on-device
transposes are needed; the host pre-transposes x):
  xT  [768, 2048]  bf16   (host-transposed activation)
  QT/KT = W.T @ xT -> [64, 2048] per head (stored as 3 pair-tiles [128, 2048])
  V = xT.T @ Wv -> [2048, 384] natural (stored per k-tile [128, 6, 65];
      column 65 of each head slot is a constant 1.0 so the PV matmul also
      accumulates the softmax denominator as output row 64)
  scoresT tiles [k=128, q=512] = KT_tile.T @ QT_chunk (PSUM), causal
      handled by narrowing the q-range and a -30000 additive mask matmul
      (identity stationary) on diagonal blocks
  softmax without max-subtraction (scores here are O(1); exp cannot
      overflow): P = exp(s/8) / sum_k exp(s/8)
  z^T unnormalized accumulated over k-tiles in PSUM [65, 512]; row 64 is
      the denominator. Normalization: reciprocal -> K=1 broadcast matmul
      -> elementwise multiply, written to zT bf16.
  out = sum_pairs zT_pair.T @ WO_pair -> [2048, 768] fp32, DMA'd out.

Biases: b_K provably cancels in softmax (it shifts every score in a row
by the same amount). b_V and b_O contribute sum_h b_V[h] @ W_O[h] + b_O,
a constant row added host-side. A nonzero b_Q would need a device-side
per-key score offset; inputs here always have b_Q = 0, so that case (and
any unexpected shape) falls back to a numpy reference implementation.
"""
import os
import sys
from collections import deque

sys.path.insert(0, "/opt/trn_rl_repo")

import numpy as np
import ml_dtypes

D_MODEL, N_HEADS, D_HEAD = 768, 12, 64
BATCH, SEQ = 4, 2048
HPG = 6           # heads per group (per core)
NPAIR = HPG // 2  # head pairs per core
NCORES = 8
QC = 512          # q chunk (moving operand width)
KT_TILES = SEQ // 128
QC_TILES = SEQ // QC
MT = D_MODEL // 128  # contraction tiles for projections
BF16 = ml_dtypes.bfloat16

_prog_cache = {}


def _numpy_ref(normalized_resid_pre, W_Q, W_K, W_V, W_O, b_Q, b_K, b_V, b_O):
    x = normalized_resid_pre.astype(np.float32)
    Q = np.einsum("bsm,hmd->bshd", x, W_Q) + b_Q
    K = np.einsum("bsm,hmd->bshd", x, W_K) + b_K
    V = np.einsum("bsm,hmd->bshd", x, W_V) + b_V
    scores = np.einsum("bqhd,bkhd->bhqk", Q, K) / np.sqrt(np.float32(W_Q.shape[-1]))
    s = x.shape[1]
    causal = np.tril(np.ones((s, s), dtype=bool))
    scores = np.where(causal, scores, -np.inf)
    scores -= scores.max(axis=-1, keepdims=True)
    e = np.exp(scores)
    probs = e / e.sum(axis=-1, keepdims=True)
    z = np.einsum("bkhd,bhqk->bqhd", V, probs)
    return (np.einsum("bqhd,hdm->bqm", z, W_O) + b_O).astype(np.float32)


def _build_program():
    from concourse import bacc, tile
    import concourse.bass as bass
    import concourse.mybir as mybir

    f32 = mybir.dt.float32
    bf16 = mybir.dt.bfloat16

    nc = bacc.Bacc(None)
    xT_d = nc.dram_tensor("xT", [D_MODEL, SEQ], bf16, kind="ExternalInput")
    wq_d = nc.dram_tensor("wq", [D_MODEL, HPG * D_HEAD], bf16, kind="ExternalInput")
    wk_d = nc.dram_tensor("wk", [D_MODEL, HPG * D_HEAD], bf16, kind="ExternalInput")
    wv_d = nc.dram_tensor("wv", [D_MODEL, HPG * D_HEAD], bf16, kind="ExternalInput")
    wo_d = nc.dram_tensor("wo", [HPG * D_HEAD, D_MODEL], bf16, kind="ExternalInput")
    mask_d = nc.dram_tensor("mask", [128, 128], bf16, kind="ExternalInput")
    ident_d = nc.dram_tensor("ident", [128, 128], bf16, kind="ExternalInput")
    out_d = nc.dram_tensor("out", [SEQ, D_MODEL], f32, kind="ExternalOutput")
    recip_d = nc.dram_tensor("recip_scratch", [HPG * (SEQ // QC), QC], bf16)

    with tile.TileContext(nc) as tc:
        with (
            tc.tile_pool(name="persist", bufs=1) as persist,
            tc.tile_pool(name="expsb", bufs=4) as expsb,
            tc.tile_pool(name="rbsb", bufs=3) as rbsb,
            tc.tile_pool(name="outsb", bufs=3) as outsb,
            tc.tile_pool(name="dtmpsb", bufs=3) as dtmpsb,
            tc.tile_pool(name="ps_big", bufs=3, space="PSUM") as ps_big,
            tc.tile_pool(name="ps_z", bufs=2, space="PSUM") as ps_z,
        )    :
            # ---- persistent SBUF tiles ----
            xT = [persist.tile([128, SEQ], bf16, tag=f"xT{i}", name=f"xT{i}") for i in range(MT)]
            wq = [persist.tile([128, HPG * D_HEAD], bf16, tag=f"wq{i}", name=f"wq{i}") for i in range(MT)]
            wk = [persist.tile([128, HPG * D_HEAD], bf16, tag=f"wk{i}", name=f"wk{i}") for i in range(MT)]
            wv = [persist.tile([128, HPG * D_HEAD], bf16, tag=f"wv{i}", name=f"wv{i}") for i in range(MT)]
            wo = [persist.tile([128, D_MODEL], bf16, tag=f"wo{i}", name=f"wo{i}") for i in range(NPAIR)]
            QTz = [persist.tile([128, SEQ], bf16, tag=f"QTz{i}", name=f"QTz{i}") for i in range(HPG)]
            KT = [persist.tile([128, SEQ], bf16, tag=f"KT{i}", name=f"KT{i}") for i in range(NPAIR)]
            zT = [persist.tile([128, SEQ], bf16, tag=f"zT{i}", name=f"zT{i}") for i in range(NPAIR)]
            V = [persist.tile([128, HPG, D_HEAD + 1], bf16, tag=f"V{i}", name=f"V{i}") for i in range(KT_TILES)]
            mask = persist.tile([128, 128], bf16, tag="mask")
            ident = persist.tile([128, 128], bf16, tag="ident")
            denom_j = [persist.tile([HPG, QC], f32, tag=f"denom{j}", name=f"denom{j}") for j in range(QC_TILES)]
            recip_j = [persist.tile([HPG, QC], bf16, tag=f"recip{j}", name=f"recip{j}") for j in range(QC_TILES)]

            # ---- input DMAs ----
            for i in range(MT):
                nc.sync.dma_start(out=xT[i], in_=xT_d[128 * i : 128 * (i + 1), :])
            for i in range(MT):
                nc.sync.dma_start(out=wq[i], in_=wq_d[128 * i : 128 * (i + 1), :])
                nc.sync.dma_start(out=wk[i], in_=wk_d[128 * i : 128 * (i + 1), :])
                nc.sync.dma_start(out=wv[i], in_=wv_d[128 * i : 128 * (i + 1), :])
            for p in range(NPAIR):
                nc.sync.dma_start(out=wo[p], in_=wo_d[128 * p : 128 * (p + 1), :])
            nc.sync.dma_start(out=mask, in_=mask_d[:, :])
            nc.sync.dma_start(out=ident, in_=ident_d[:, :])
            for h in range(HPG):
                r0 = 64 * (h % 2)
                nc.vector.memset(QTz[h][64 - r0 : 128 - r0, :], 0.0)
            for kt in range(KT_TILES):
                nc.vector.memset(V[kt][:, :, D_HEAD : D_HEAD + 1], 1.0)

            # ---- phase 1: projections ----
            for p in range(NPAIR):
                cols = slice(128 * p, 128 * (p + 1))
                for j in range(QC_TILES):
                    qs = slice(QC * j, QC * (j + 1))
                    psq = ps_big.tile([128, QC], f32, tag="big")
                    for m in range(MT):
                        nc.tensor.matmul(psq, lhsT=wq[m][:, cols], rhs=xT[m][:, qs],
                                         start=(m == 0), stop=(m == MT - 1))
                    nc.scalar.activation(out=QTz[2 * p][0:64, qs], in_=psq[0:64, :],
                                         func=mybir.ActivationFunctionType.Copy)
                    nc.scalar.activation(out=QTz[2 * p + 1][64:128, qs], in_=psq[64:128, :],
                                         func=mybir.ActivationFunctionType.Copy)
                    psk = ps_big.tile([128, QC], f32, tag="big")
                    for m in range(MT):
                        nc.tensor.matmul(psk, lhsT=wk[m][:, cols], rhs=xT[m][:, qs],
                                         start=(m == 0), stop=(m == MT - 1))
                    nc.scalar.activation(out=KT[p][:, qs], in_=psk,
                                         func=mybir.ActivationFunctionType.Copy)
            for kt in range(KT_TILES):
                ks = slice(128 * kt, 128 * (kt + 1))
                psv = ps_big.tile([128, HPG * D_HEAD], f32, tag="big")
                for m in range(MT):
                    nc.tensor.matmul(psv, lhsT=xT[m][:, ks], rhs=wv[m],
                                     start=(m == 0), stop=(m == MT - 1))
                nc.scalar.activation(
                    out=V[kt][:, :, 0:D_HEAD],
                    in_=psv.rearrange("p (h d) -> p h d", h=HPG),
                    func=mybir.ActivationFunctionType.Copy,
                )

            # ---- phase 2: attention (j outer so normalization + output
            # projection for q-block j pipeline behind attention of j+1) ----
            # zT first holds the unnormalized z^T; denominators for the 6
            # heads of one q-block collect into denom_j[j] (partition 0..5)
            # so one partition-parallel reciprocal covers the block (a
            # [1, 512] DVE reciprocal is serial, ~3.4us each).
            def emit_scores(h, j, kt2):
                p = h // 2
                pss = ps_big.tile([128, 2 * QC], f32, tag="big", name="pss")
                off0 = 0
                for u in (0, 1):
                    kt = kt2 + u
                    delta = kt - 4 * j  # >=0 on diagonal blocks
                    off = 128 * delta if delta >= 0 else 0
                    if u == 0:
                        off0 = off
                    nc.tensor.matmul(
                        pss[:, QC * u + off : QC * (u + 1)],
                        lhsT=KT[p][:, 128 * kt : 128 * (kt + 1)],
                        rhs=QTz[h][:, QC * j + off : QC * (j + 1)],
                        start=True, stop=(delta < 0),
                        skip_group_check=True,
                    )
                    if delta >= 0:
                        nc.tensor.matmul(
                            pss[:, QC * u + off : QC * u + off + 128],
                            lhsT=ident, rhs=mask,
                            start=False, stop=True,
                            skip_group_check=True,
                        )
                expt = expsb.tile([128, 2 * QC], bf16, tag="exp", name="expt")
                nc.scalar.activation(out=expt[:, off0:], in_=pss[:, off0:],
                                     func=mybir.ActivationFunctionType.Exp,
                                     scale=0.125)
                return expt

            def emit_pv(h, j, psz, nkt, kt2, expt):
                for u in (0, 1):
                    kt = kt2 + u
                    delta = kt - 4 * j
                    off = 128 * delta if delta >= 0 else 0
                    nc.tensor.matmul(
                        psz[:, off:QC],
                        lhsT=V[kt][:, h, :],
                        rhs=expt[:, QC * u + off : QC * (u + 1)],
                        start=(kt == 0), stop=(kt == nkt - 1),
                        skip_group_check=True,
                    )

            def make_norm_outproj(j):
                # normalization + output projection for q-block j; its ~13us
                # of DVE/DMA latency is emitted AFTER the next block's first
                # attention heads so the in-order PE queue never drains on it
                def emit():
                    qs = slice(QC * j, QC * (j + 1))
                    rtmp = dtmpsb.tile([HPG, QC], f32, tag="rtmp", name="rtmp")
                    nc.vector.reciprocal_approx_fast(rtmp, denom_j[j])
                    nc.vector.tensor_copy(recip_j[j], rtmp)
                    for h in range(HPG):
                        row = HPG * j + h
                        nc.sync.dma_start(out=recip_d[row : row + 1, :],
                                          in_=recip_j[j][h : h + 1, :])
                    for h in range(HPG):
                        p, r0 = h // 2, 64 * (h % 2)
                        row = HPG * j + h
                        sl = recip_d[row : row + 1, :]
                        rb = rbsb.tile([128, QC], bf16, tag="rb", name="rb")
                        nc.sync.dma_start(
                            out=rb[r0 : r0 + 64, :],
                            in_=bass.AP(tensor=sl.tensor, offset=sl.offset,
                                        ap=[[0, D_HEAD]] + list(sl.ap[-1:])))
                        nc.vector.tensor_mul(zT[p][r0 : r0 + 64, qs],
                                             zT[p][r0 : r0 + 64, qs],
                                             rb[r0 : r0 + 64, :])
                    for c in range(4 * j, 4 * (j + 1)):
                        cs = slice(128 * c, 128 * (c + 1))
                        pso = ps_big.tile([128, D_MODEL], f32, tag="big", name="pso")
                        for p in range(NPAIR):
                            nc.tensor.matmul(pso[:, 0:512], lhsT=zT[p][:, cs],
                                             rhs=wo[p][:, 0:512],
                                             start=(p == 0), stop=(p == NPAIR - 1))
                            nc.tensor.matmul(pso[:, 512:768], lhsT=zT[p][:, cs],
                                             rhs=wo[p][:, 512:768],
                                             start=(p == 0), stop=(p == NPAIR - 1))
                        outt = outsb.tile([128, D_MODEL], f32, tag="out", name="outt")
                        nc.vector.tensor_copy(outt, pso)
                        nc.sync.dma_start(out=out_d[cs, :], in_=outt)
                return emit

            deferred = None
            for j in range(QC_TILES):
                qs = slice(QC * j, QC * (j + 1))
                nkt = 4 * j + 4  # k-tiles this q-chunk needs (always even)
                for h in range(HPG):
                    p, r0 = h // 2, 64 * (h % 2)
                    psz = ps_z.tile([D_HEAD + 1, QC], f32, tag="z")
                    # scores run two k-pairs ahead of PV so the tensor engine
                    # has work while ACT computes the exp of previous pairs
                    pend = deque()
                    for kt2 in range(0, nkt, 2):
                        expt = emit_scores(h, j, kt2)
                        pend.append((kt2, expt))
                        if len(pend) > 2:
                            kt2p, exptp = pend.popleft()
                            emit_pv(h, j, psz, nkt, kt2p, exptp)
                    while pend:
                        kt2p, exptp = pend.popleft()
                        emit_pv(h, j, psz, nkt, kt2p, exptp)

                    dtmp = dtmpsb.tile([1, QC], f32, tag="dtmp", name="dtmp")
                    nc.vector.tensor_copy(dtmp, psz[D_HEAD : D_HEAD + 1, :])
                    nc.sync.dma_start(out=denom_j[j][h : h + 1, :], in_=dtmp)
                    nc.vector.tensor_copy(zT[p][r0 : r0 + 64, qs], psz[0:D_HEAD, :])

                    if deferred is not None and h == 1:
                        deferred()
                        deferred = None
                deferred = make_norm_outproj(j)
            deferred()

    nc.finalize()
    return nc


def kernel(**inputs):
    x = inputs["normalized_resid_pre"]
    W_Q, W_K, W_V, W_O = inputs["W_Q"], inputs["W_K"], inputs["W_V"], inputs["W_O"]
    b_Q, b_K, b_V, b_O = inputs["b_Q"], inputs["b_K"], inputs["b_V"], inputs["b_O"]

    expected = (
        x.shape == (BATCH, SEQ, D_MODEL)
        and W_Q.shape == (N_HEADS, D_MODEL, D_HEAD)
        and W_K.shape == (N_HEADS, D_MODEL, D_HEAD)
        and W_V.shape == (N_HEADS, D_MODEL, D_HEAD)
        and W_O.shape == (N_HEADS, D_HEAD, D_MODEL)
        and not np.any(b_Q)
    )
    if not expected:
        return _numpy_ref(**inputs)

    from concourse.bass_utils import run_bass_kernel_spmd

    if "nc" not in _prog_cache:
        _prog_cache["nc"] = _build_program()
    nc = _prog_cache["nc"]

    # host-side prep: transpose + cast + pack per head-group
    xT = np.ascontiguousarray(x.transpose(0, 2, 1)).astype(BF16)  # [B, 768, 2048]
    # b_K shifts every score in a softmax row equally -> cancels exactly.
    groups = []
    for g in range(2):
        hs = slice(HPG * g, HPG * (g + 1))
        groups.append({
            "wq": np.ascontiguousarray(W_Q[hs].transpose(1, 0, 2).reshape(D_MODEL, HPG * D_HEAD)).astype(BF16),
            "wk": np.ascontiguousarray(W_K[hs].transpose(1, 0, 2).reshape(D_MODEL, HPG * D_HEAD)).astype(BF16),
            "wv": np.ascontiguousarray(W_V[hs].transpose(1, 0, 2).reshape(D_MODEL, HPG * D_HEAD)).astype(BF16),
            "wo": np.ascontiguousarray(W_O[hs].reshape(HPG * D_HEAD, D_MODEL)).astype(BF16),
        })
    ii, jj = np.arange(128)[:, None], np.arange(128)[None, :]
    mask = np.where(jj >= ii, np.float32(0.0), np.float32(-30000.0)).astype(BF16)
    ident = np.eye(128, dtype=np.float32).astype(BF16)

    in_maps = []
    for c in range(NCORES):
        b, g = c // 2, c % 2
        m = {"xT": xT[b], "mask": mask, "ident": ident}
        m.update(groups[g])
        in_maps.append(m)

    trace = bool(os.environ.get("ATTN_KERNEL_TRACE"))
    res = run_bass_kernel_spmd(nc, in_maps, list(range(NCORES)), trace=trace)
    _prog_cache["last_exec_time_ns"] = res.exec_time_ns
    _prog_cache["last_results"] = res

    # b_V/b_O fold into a constant row (softmax weights sum to 1).
    const_row = np.einsum("hd,hdm->m", b_V.astype(np.float64), W_O.astype(np.float64))
    const_row = (const_row + b_O.astype(np.float64)).astype(np.float32)

    out = np.empty((BATCH, SEQ, D_MODEL), dtype=np.float32)
    for b in range(BATCH):
        out[b] = res.results[2 * b]["out"] + res.results[2 * b + 1]["out"] + const_row
    return out


# revision 31
# speedup vs baseline: 1.2361x; 1.0852x over previous
"""Multi-head causal attention (dense_transformer) on 8 trn2 NeuronCores.

Problem: x[4, 2048, 768], 12 heads of d_head=64, causal softmax, out proj.

Sharding: data-parallel over batch (4) x tensor-parallel over heads
(2 groups of 6). Core c handles (batch c//2, heads 6*(c%2)..6*(c%2)+5) and
returns its partial output sum over its heads; the host adds the two
partials per batch ("all-reduce" of size 2 done host-side).

Device kernel layout (everything lives transposed so no on-device
transposes are needed; the host pre-transposes x):
  xT  [768, 2048]  bf16   (host-transposed activation)
  QT/KT = W.T @ xT -> [64, 2048] per head (stored as 3 pair-tiles [128, 2048])
  V = xT.T @ Wv -> [2048, 384] natural (stored per k-tile [128, 6, 65];
      column 65 of each head slot is a constant 1.0 so the PV matmul also
      accumulates the softmax denominator as output row 64)
  scoresT tiles [k=128, q=512] = KT_tile.T @ QT_chunk (PSUM), causal
      handled by narrowing the q-range and a -30000 additive mask matmul
      (identity stationary) on diagonal blocks
  softmax without max-subtraction (scores here are O(1); exp cannot
      overflow): P = exp(s/8) / sum_k exp(s/8)
  z^T unnormalized accumulated over k-tiles in PSUM [65, 512]; row 64 is
      the denominator. Normalization: reciprocal -> K=1 broadcast matmul
      -> elementwise multiply, written to zT bf16.
  out = sum_pairs zT_pair.T @ WO_pair -> [2048, 768] fp32, DMA'd out.

Biases: b_K provably cancels in softmax (it shifts every score in a row
by the same amount). b_V and b_O contribute sum_h b_V[h] @ W_O[h] + b_O,
a constant row added host-side. A nonzero b_Q would need a device-side
per-key score offset; inputs here always have b_Q = 0, so that case (and
any unexpected shape) falls back to a numpy reference implementation.
"""
import os
import sys
from collections import deque

sys.path.insert(0, "/opt/trn_rl_repo")

import numpy as np
import ml_dtypes

D_MODEL, N_HEADS, D_HEAD = 768, 12, 64
BATCH, SEQ = 4, 2048
HPG = 6           # heads per group (per core)
NPAIR = HPG // 2  # head pairs per core
NCORES = 8
QC = 512          # q chunk (moving operand width)
KT_TILES = SEQ // 128
QC_TILES = SEQ // QC
MT = D_MODEL // 128  # contraction tiles for projections
BF16 = ml_dtypes.bfloat16

_prog_cache = {}


def _numpy_ref(normalized_resid_pre, W_Q, W_K, W_V, W_O, b_Q, b_K, b_V, b_O):
    x = normalized_resid_pre.astype(np.float32)
    Q = np.einsum("bsm,hmd->bshd", x, W_Q) + b_Q
    K = np.einsum("bsm,hmd->bshd", x, W_K) + b_K
    V = np.einsum("bsm,hmd->bshd", x, W_V) + b_V
    scores = np.einsum("bqhd,bkhd->bhqk", Q, K) / np.sqrt(np.float32(W_Q.shape[-1]))
    s = x.shape[1]
    causal = np.tril(np.ones((s, s), dtype=bool))
    scores = np.where(causal, scores, -np.inf)
    scores -= scores.max(axis=-1, keepdims=True)
    e = np.exp(scores)
    probs = e / e.sum(axis=-1, keepdims=True)
    z = np.einsum("bkhd,bhqk->bqhd", V, probs)
    return (np.einsum("bqhd,hdm->bqm", z, W_O) + b_O).astype(np.float32)


def _build_program():
    from concourse import bacc, tile
    import concourse.bass as bass
    import concourse.mybir as mybir

    f32 = mybir.dt.float32
    bf16 = mybir.dt.bfloat16

    nc = bacc.Bacc(None)
    xT_d = nc.dram_tensor("xT", [D_MODEL, SEQ], bf16, kind="ExternalInput")
    wq_d = nc.dram_tensor("wq", [D_MODEL, HPG * D_HEAD], bf16, kind="ExternalInput")
    wk_d = nc.dram_tensor("wk", [D_MODEL, HPG * D_HEAD], bf16, kind="ExternalInput")
    wv_d = nc.dram_tensor("wv", [D_MODEL, HPG * D_HEAD], bf16, kind="ExternalInput")
    wo_d = nc.dram_tensor("wo", [HPG * D_HEAD, D_MODEL], bf16, kind="ExternalInput")
    mask_d = nc.dram_tensor("mask", [128, 128], bf16, kind="ExternalInput")
    out_d = nc.dram_tensor("out", [SEQ, D_MODEL], f32, kind="ExternalOutput")
    recip_d = nc.dram_tensor("recip_scratch", [HPG * (SEQ // QC), QC], f32)

    with tile.TileContext(nc) as tc:
        with (
            tc.tile_pool(name="persist", bufs=1) as persist,
            tc.tile_pool(name="expsb", bufs=4) as expsb,
            tc.tile_pool(name="rbsb", bufs=3) as rbsb,
            tc.tile_pool(name="outsb", bufs=3) as outsb,
            tc.tile_pool(name="dtmpsb", bufs=3) as dtmpsb,
            tc.tile_pool(name="ps_big", bufs=3, space="PSUM") as ps_big,
            tc.tile_pool(name="ps_z", bufs=2, space="PSUM") as ps_z,
        )    :
            # ---- persistent SBUF tiles ----
            xT = [persist.tile([128, SEQ], bf16, tag=f"xT{i}", name=f"xT{i}") for i in range(MT)]
            wq = [persist.tile([128, HPG * D_HEAD], bf16, tag=f"wq{i}", name=f"wq{i}") for i in range(MT)]
            wk = [persist.tile([128, HPG * D_HEAD], bf16, tag=f"wk{i}", name=f"wk{i}") for i in range(MT)]
            wv = [persist.tile([128, HPG * D_HEAD], bf16, tag=f"wv{i}", name=f"wv{i}") for i in range(MT)]
            wo = [persist.tile([128, D_MODEL], bf16, tag=f"wo{i}", name=f"wo{i}") for i in range(NPAIR)]
            QTz = [persist.tile([128, SEQ], bf16, tag=f"QTz{i}", name=f"QTz{i}") for i in range(HPG)]
            KT = [persist.tile([128, SEQ], bf16, tag=f"KT{i}", name=f"KT{i}") for i in range(NPAIR)]
            zT = [persist.tile([128, SEQ], bf16, tag=f"zT{i}", name=f"zT{i}") for i in range(NPAIR)]
            V = [persist.tile([128, HPG, D_HEAD + 1], bf16, tag=f"V{i}", name=f"V{i}") for i in range(KT_TILES)]
            mask01 = persist.tile([128, 128], bf16, tag="mask01")

            # ---- input DMAs ----
            for i in range(MT):
                nc.sync.dma_start(out=xT[i], in_=xT_d[128 * i : 128 * (i + 1), :])
            for i in range(MT):
                nc.sync.dma_start(out=wq[i], in_=wq_d[128 * i : 128 * (i + 1), :])
                nc.sync.dma_start(out=wk[i], in_=wk_d[128 * i : 128 * (i + 1), :])
                nc.sync.dma_start(out=wv[i], in_=wv_d[128 * i : 128 * (i + 1), :])
            for p in range(NPAIR):
                nc.sync.dma_start(out=wo[p], in_=wo_d[128 * p : 128 * (p + 1), :])
            nc.sync.dma_start(out=mask01, in_=mask_d[:, :])
            for h in range(HPG):
                r0 = 64 * (h % 2)
                nc.vector.memset(QTz[h][64 - r0 : 128 - r0, :], 0.0)
            for kt in range(KT_TILES):
                nc.vector.memset(V[kt][:, :, D_HEAD : D_HEAD + 1], 1.0)

            # ---- emission helpers ----
            def emit_qkv_pair(p):
                cols = slice(128 * p, 128 * (p + 1))
                for j in range(QC_TILES):
                    qs = slice(QC * j, QC * (j + 1))
                    psq = ps_big.tile([128, QC], f32, tag="big", name="psq")
                    for m in range(MT):
                        nc.tensor.matmul(psq, lhsT=wq[m][:, cols], rhs=xT[m][:, qs],
                                         start=(m == 0), stop=(m == MT - 1))
                    nc.scalar.activation(out=QTz[2 * p][0:64, qs], in_=psq[0:64, :],
                                         func=mybir.ActivationFunctionType.Copy)
                    nc.scalar.activation(out=QTz[2 * p + 1][64:128, qs], in_=psq[64:128, :],
                                         func=mybir.ActivationFunctionType.Copy)
                    psk = ps_big.tile([128, QC], f32, tag="big", name="psk")
                    for m in range(MT):
                        nc.tensor.matmul(psk, lhsT=wk[m][:, cols], rhs=xT[m][:, qs],
                                         start=(m == 0), stop=(m == MT - 1))
                    nc.scalar.activation(out=KT[p][:, qs], in_=psk,
                                         func=mybir.ActivationFunctionType.Copy)

            def emit_v(kts):
                for kt in kts:
                    ks = slice(128 * kt, 128 * (kt + 1))
                    psv = ps_big.tile([128, HPG * D_HEAD], f32, tag="big", name="psv")
                    for m in range(MT):
                        nc.tensor.matmul(psv, lhsT=xT[m][:, ks], rhs=wv[m],
                                         start=(m == 0), stop=(m == MT - 1))
                    nc.scalar.activation(
                        out=V[kt][:, :, 0:D_HEAD],
                        in_=psv.rearrange("p (h d) -> p h d", h=HPG),
                        func=mybir.ActivationFunctionType.Copy,
                    )

            def emit_scores(h, j, kt2):
                p = h // 2
                pss = ps_big.tile([128, 2 * QC], f32, tag="big", name="pss")
                off0 = 0
                for u in (0, 1):
                    kt = kt2 + u
                    delta = kt - 4 * j  # >=0 on diagonal blocks
                    off = 128 * delta if delta >= 0 else 0
                    if u == 0:
                        off0 = off
                    nc.tensor.matmul(
                        pss[:, QC * u + off : QC * (u + 1)],
                        lhsT=KT[p][:, 128 * kt : 128 * (kt + 1)],
                        rhs=QTz[h][:, QC * j + off : QC * (j + 1)],
                        start=True, stop=True,
                        skip_group_check=True,
                    )
                expt = expsb.tile([128, 2 * QC], bf16, tag="exp", name="expt")
                nc.scalar.activation(out=expt[:, off0:], in_=pss[:, off0:],
                                     func=mybir.ActivationFunctionType.Exp,
                                     scale=0.125)
                for u in (0, 1):
                    delta = kt2 + u - 4 * j
                    if delta >= 0:
                        off = 128 * delta
                        blk = slice(QC * u + off, QC * u + off + 128)
                        nc.vector.tensor_mul(expt[:, blk], expt[:, blk], mask01)
                return expt

            def emit_pv(h, j, psz, nkt, kt2, expt):
                for u in (0, 1):
                    kt = kt2 + u
                    delta = kt - 4 * j
                    off = 128 * delta if delta >= 0 else 0
                    nc.tensor.matmul(
                        psz[:, off:QC],
                        lhsT=V[kt][:, h, :],
                        rhs=expt[:, QC * u + off : QC * (u + 1)],
                        start=(kt == 0), stop=(kt == nkt - 1),
                        skip_group_check=True,
                    )

            def emit_attention(h, j):
                # k-loop with scores staggered two k-pairs ahead of PV, then
                # the per-head normalization chain (approx reciprocal -> DRAM
                # hop -> partition-broadcast DMA -> multiply) which hides
                # behind subsequent attention work
                p, r0 = h // 2, 64 * (h % 2)
                qs = slice(QC * j, QC * (j + 1))
                nkt = 4 * j + 4
                psz = ps_z.tile([D_HEAD + 1, QC], f32, tag="z", name="psz")
                pend = deque()
                for kt2 in range(0, nkt, 2):
                    expt = emit_scores(h, j, kt2)
                    pend.append((kt2, expt))
                    if len(pend) > 2:
                        kt2p, exptp = pend.popleft()
                        emit_pv(h, j, psz, nkt, kt2p, exptp)
                while pend:
                    kt2p, exptp = pend.popleft()
                    emit_pv(h, j, psz, nkt, kt2p, exptp)

                row = HPG * j + h
                dtmp = dtmpsb.tile([1, QC], f32, tag="dtmp", name="dtmp")
                nc.vector.tensor_copy(dtmp, psz[D_HEAD : D_HEAD + 1, :])
                rtmp = dtmpsb.tile([1, QC], f32, tag="rtmp", name="rtmp")
                nc.vector.reciprocal_approx_fast(rtmp, dtmp)
                nc.sync.dma_start(out=recip_d[row : row + 1, :], in_=rtmp)
                nc.vector.tensor_copy(zT[p][r0 : r0 + 64, qs], psz[0:D_HEAD, :])
                sl = recip_d[row : row + 1, :]
                rb = rbsb.tile([128, QC], f32, tag="rb", name="rb")
                nc.sync.dma_start(
                    out=rb[r0 : r0 + 64, :],
                    in_=bass.AP(tensor=sl.tensor, offset=sl.offset,
                                ap=[[0, D_HEAD]] + list(sl.ap[-1:])))
                nc.vector.tensor_mul(zT[p][r0 : r0 + 64, qs],
                                     zT[p][r0 : r0 + 64, qs],
                                     rb[r0 : r0 + 64, :])

            def emit_outproj(j):
                for c in range(4 * j, 4 * (j + 1)):
                    cs = slice(128 * c, 128 * (c + 1))
                    pso = ps_big.tile([128, D_MODEL], f32, tag="big", name="pso")
                    for p in range(NPAIR):
                        nc.tensor.matmul(pso[:, 0:512], lhsT=zT[p][:, cs],
                                         rhs=wo[p][:, 0:512],
                                         start=(p == 0), stop=(p == NPAIR - 1))
                        nc.tensor.matmul(pso[:, 512:768], lhsT=zT[p][:, cs],
                                         rhs=wo[p][:, 512:768],
                                         start=(p == 0), stop=(p == NPAIR - 1))
                    outt = outsb.tile([128, D_MODEL], f32, tag="out", name="outt")
                    nc.vector.tensor_copy(outt, pso)
                    nc.sync.dma_start(out=out_d[cs, :], in_=outt)

            # ---- schedule: j=0 attention interleaves into the projection
            # phase (PE issues in order; exp latency of the small j=0 blocks
            # hides inside projection matmul streams) ----
            emit_qkv_pair(0)
            emit_v([0, 1, 2, 3])
            emit_attention(0, 0)
            emit_attention(1, 0)
            emit_qkv_pair(1)
            emit_v([4, 5, 6, 7])
            emit_attention(2, 0)
            emit_attention(3, 0)
            emit_qkv_pair(2)
            emit_v([8, 9, 10, 11])
            emit_attention(4, 0)
            emit_attention(5, 0)
            emit_v([12, 13, 14, 15])
            for j in range(1, QC_TILES):
                for h in range(HPG):
                    emit_attention(h, j)
                    if h == 1:
                        emit_outproj(j - 1)
            emit_outproj(QC_TILES - 1)

    nc.finalize()
    return nc


def kernel(**inputs):
    x = inputs["normalized_resid_pre"]
    W_Q, W_K, W_V, W_O = inputs["W_Q"], inputs["W_K"], inputs["W_V"], inputs["W_O"]
    b_Q, b_K, b_V, b_O = inputs["b_Q"], inputs["b_K"], inputs["b_V"], inputs["b_O"]

    expected = (
        x.shape == (BATCH, SEQ, D_MODEL)
        and W_Q.shape == (N_HEADS, D_MODEL, D_HEAD)
        and W_K.shape == (N_HEADS, D_MODEL, D_HEAD)
        and W_V.shape == (N_HEADS, D_MODEL, D_HEAD)
        and W_O.shape == (N_HEADS, D_HEAD, D_MODEL)
        and not np.any(b_Q)
    )
    if not expected:
        return _numpy_ref(**inputs)

    from concourse.bass_utils import run_bass_kernel_spmd

    if "nc" not in _prog_cache:
        _prog_cache["nc"] = _build_program()
    nc = _prog_cache["nc"]

    # host-side prep: transpose + cast + pack per head-group
    xT = np.ascontiguousarray(x.transpose(0, 2, 1)).astype(BF16)  # [B, 768, 2048]
    # b_K shifts every score in a softmax row equally -> cancels exactly.
    groups = []
    for g in range(2):
        hs = slice(HPG * g, HPG * (g + 1))
        groups.append({
            "wq": np.ascontiguousarray(W_Q[hs].transpose(1, 0, 2).reshape(D_MODEL, HPG * D_HEAD)).astype(BF16),
            "wk": np.ascontiguousarray(W_K[hs].transpose(1, 0, 2).reshape(D_MODEL, HPG * D_HEAD)).astype(BF16),
            "wv": np.ascontiguousarray(W_V[hs].transpose(1, 0, 2).reshape(D_MODEL, HPG * D_HEAD)).astype(BF16),
            "wo": np.ascontiguousarray(W_O[hs].reshape(HPG * D_HEAD, D_MODEL)).astype(BF16),
        })
    ii, jj = np.arange(128)[:, None], np.arange(128)[None, :]
    mask = np.where(jj >= ii, np.float32(1.0), np.float32(0.0)).astype(BF16)

    in_maps = []
    for c in range(NCORES):
        b, g = c // 2, c % 2
        m = {"xT": xT[b], "mask": mask}
        m.update(groups[g])
        in_maps.append(m)

    trace = bool(os.environ.get("ATTN_KERNEL_TRACE"))
    res = run_bass_kernel_spmd(nc, in_maps, list(range(NCORES)), trace=trace)
    _prog_cache["last_exec_time_ns"] = res.exec_time_ns
    _prog_cache["last_results"] = res

    # b_V/b_O fold into a constant row (softmax weights sum to 1).
    const_row = np.einsum("hd,hdm->m", b_V.astype(np.float64), W_O.astype(np.float64))
    const_row = (const_row + b_O.astype(np.float64)).astype(np.float32)

    out = np.empty((BATCH, SEQ, D_MODEL), dtype=np.float32)
    for b in range(BATCH):
        out[b] = res.results[2 * b]["out"] + res.results[2 * b + 1]["out"] + const_row
    return out


# revision 32
# speedup vs baseline: 1.2395x; 1.0027x over previous
"""Multi-head causal attention (dense_transformer) on 8 trn2 NeuronCores.

Problem: x[4, 2048, 768], 12 heads of d_head=64, causal softmax, out proj.

Sharding: data-parallel over batch (4) x tensor-parallel over heads
(2 groups of 6). Core c handles (batch c//2, heads 6*(c%2)..6*(c%2)+5) and
returns its partial output sum over its heads; the host adds the two
partials per batch ("all-reduce" of size 2 done host-side).

Device kernel layout (everything lives transposed so no on-device
transposes are needed; the host pre-transposes x):
  xT  [768, 2048]  bf16   (host-transposed activation)
  QT/KT = W.T @ xT -> [64, 2048] per head (stored as 3 pair-tiles [128, 2048])
  V = xT.T @ Wv -> [2048, 384] natural (stored per k-tile [128, 6, 65];
      column 65 of each head slot is a constant 1.0 so the PV matmul also
      accumulates the softmax denominator as output row 64)
  scoresT tiles [k=128, q=512] = KT_tile.T @ QT_chunk (PSUM), causal
      handled by narrowing the q-range and a -30000 additive mask matmul
      (identity stationary) on diagonal blocks
  softmax without max-subtraction (scores here are O(1); exp cannot
      overflow): P = exp(s/8) / sum_k exp(s/8)
  z^T unnormalized accumulated over k-tiles in PSUM [65, 512]; row 64 is
      the denominator. Normalization: reciprocal -> K=1 broadcast matmul
      -> elementwise multiply, written to zT bf16.
  out = sum_pairs zT_pair.T @ WO_pair -> [2048, 768] fp32, DMA'd out.

Biases: b_K provably cancels in softmax (it shifts every score in a row
by the same amount). b_V and b_O contribute sum_h b_V[h] @ W_O[h] + b_O,
a constant row added host-side. A nonzero b_Q would need a device-side
per-key score offset; inputs here always have b_Q = 0, so that case (and
any unexpected shape) falls back to a numpy reference implementation.
"""
import os
import sys
from collections import deque

sys.path.insert(0, "/opt/trn_rl_repo")

import numpy as np
import ml_dtypes

D_MODEL, N_HEADS, D_HEAD = 768, 12, 64
BATCH, SEQ = 4, 2048
HPG = 6           # heads per group (per core)
NPAIR = HPG // 2  # head pairs per core
NCORES = 8
QC = 512          # q chunk (moving operand width)
KT_TILES = SEQ // 128
QC_TILES = SEQ // QC
MT = D_MODEL // 128  # contraction tiles for projections
BF16 = ml_dtypes.bfloat16

_prog_cache = {}


def _numpy_ref(normalized_resid_pre, W_Q, W_K, W_V, W_O, b_Q, b_K, b_V, b_O):
    x = normalized_resid_pre.astype(np.float32)
    Q = np.einsum("bsm,hmd->bshd", x, W_Q) + b_Q
    K = np.einsum("bsm,hmd->bshd", x, W_K) + b_K
    V = np.einsum("bsm,hmd->bshd", x, W_V) + b_V
    scores = np.einsum("bqhd,bkhd->bhqk", Q, K) / np.sqrt(np.float32(W_Q.shape[-1]))
    s = x.shape[1]
    causal = np.tril(np.ones((s, s), dtype=bool))
    scores = np.where(causal, scores, -np.inf)
    scores -= scores.max(axis=-1, keepdims=True)
    e = np.exp(scores)
    probs = e / e.sum(axis=-1, keepdims=True)
    z = np.einsum("bkhd,bhqk->bqhd", V, probs)
    return (np.einsum("bqhd,hdm->bqm", z, W_O) + b_O).astype(np.float32)


def _build_program():
    from concourse import bacc, tile
    import concourse.bass as bass
    import concourse.mybir as mybir

    f32 = mybir.dt.float32
    bf16 = mybir.dt.bfloat16

    nc = bacc.Bacc(None)
    xT_d = nc.dram_tensor("xT", [D_MODEL, SEQ], bf16, kind="ExternalInput")
    wq_d = nc.dram_tensor("wq", [D_MODEL, HPG * D_HEAD], bf16, kind="ExternalInput")
    wk_d = nc.dram_tensor("wk", [D_MODEL, HPG * D_HEAD], bf16, kind="ExternalInput")
    wv_d = nc.dram_tensor("wv", [D_MODEL, HPG * D_HEAD], bf16, kind="ExternalInput")
    wo_d = nc.dram_tensor("wo", [HPG * D_HEAD, D_MODEL], bf16, kind="ExternalInput")
    mask_d = nc.dram_tensor("mask", [128, 128], bf16, kind="ExternalInput")
    out_d = nc.dram_tensor("out", [SEQ, D_MODEL], f32, kind="ExternalOutput")
    recip_d = nc.dram_tensor("recip_scratch", [HPG * (SEQ // QC), QC], f32)

    with tile.TileContext(nc) as tc:
        with (
            tc.tile_pool(name="persist", bufs=1) as persist,
            tc.tile_pool(name="expsb", bufs=4) as expsb,
            tc.tile_pool(name="rbsb", bufs=3) as rbsb,
            tc.tile_pool(name="outsb", bufs=3) as outsb,
            tc.tile_pool(name="dtmpsb", bufs=3) as dtmpsb,
            tc.tile_pool(name="ps_big", bufs=3, space="PSUM") as ps_big,
            tc.tile_pool(name="ps_z", bufs=2, space="PSUM") as ps_z,
        )    :
            # ---- persistent SBUF tiles ----
            xT = [persist.tile([128, SEQ], bf16, tag=f"xT{i}", name=f"xT{i}") for i in range(MT)]
            wq = [persist.tile([128, HPG * D_HEAD], bf16, tag=f"wq{i}", name=f"wq{i}") for i in range(MT)]
            wk = [persist.tile([128, HPG * D_HEAD], bf16, tag=f"wk{i}", name=f"wk{i}") for i in range(MT)]
            wv = [persist.tile([128, HPG * D_HEAD], bf16, tag=f"wv{i}", name=f"wv{i}") for i in range(MT)]
            wo = [persist.tile([128, D_MODEL], bf16, tag=f"wo{i}", name=f"wo{i}") for i in range(NPAIR)]
            QTz = [persist.tile([128, SEQ], bf16, tag=f"QTz{i}", name=f"QTz{i}") for i in range(HPG)]
            KT = [persist.tile([128, SEQ], bf16, tag=f"KT{i}", name=f"KT{i}") for i in range(NPAIR)]
            zT = [persist.tile([128, SEQ], bf16, tag=f"zT{i}", name=f"zT{i}") for i in range(NPAIR)]
            V = [persist.tile([128, HPG, D_HEAD + 1], bf16, tag=f"V{i}", name=f"V{i}") for i in range(KT_TILES)]
            mask01 = persist.tile([128, 128], bf16, tag="mask01")

            # ---- input DMAs ----
            for i in range(MT):
                nc.sync.dma_start(out=xT[i], in_=xT_d[128 * i : 128 * (i + 1), :])
            for i in range(MT):
                nc.sync.dma_start(out=wq[i], in_=wq_d[128 * i : 128 * (i + 1), :])
                nc.sync.dma_start(out=wk[i], in_=wk_d[128 * i : 128 * (i + 1), :])
                nc.sync.dma_start(out=wv[i], in_=wv_d[128 * i : 128 * (i + 1), :])
            for p in range(NPAIR):
                nc.sync.dma_start(out=wo[p], in_=wo_d[128 * p : 128 * (p + 1), :])
            nc.sync.dma_start(out=mask01, in_=mask_d[:, :])
            for h in range(HPG):
                r0 = 64 * (h % 2)
                nc.vector.memset(QTz[h][64 - r0 : 128 - r0, :], 0.0)
            for kt in range(KT_TILES):
                nc.vector.memset(V[kt][:, :, D_HEAD : D_HEAD + 1], 1.0)

            # ---- emission helpers ----
            def emit_qkv_pair(p):
                cols = slice(128 * p, 128 * (p + 1))
                for j in range(QC_TILES):
                    qs = slice(QC * j, QC * (j + 1))
                    psq = ps_big.tile([128, QC], f32, tag="big", name="psq")
                    for m in range(MT):
                        nc.tensor.matmul(psq, lhsT=wq[m][:, cols], rhs=xT[m][:, qs],
                                         start=(m == 0), stop=(m == MT - 1))
                    nc.vector.tensor_copy(QTz[2 * p][0:64, qs], psq[0:64, :])
                    nc.vector.tensor_copy(QTz[2 * p + 1][64:128, qs], psq[64:128, :])
                    psk = ps_big.tile([128, QC], f32, tag="big", name="psk")
                    for m in range(MT):
                        nc.tensor.matmul(psk, lhsT=wk[m][:, cols], rhs=xT[m][:, qs],
                                         start=(m == 0), stop=(m == MT - 1))
                    nc.vector.tensor_copy(KT[p][:, qs], psk)

            def emit_v(kts):
                for kt in kts:
                    ks = slice(128 * kt, 128 * (kt + 1))
                    psv = ps_big.tile([128, HPG * D_HEAD], f32, tag="big", name="psv")
                    for m in range(MT):
                        nc.tensor.matmul(psv, lhsT=xT[m][:, ks], rhs=wv[m],
                                         start=(m == 0), stop=(m == MT - 1))
                    nc.vector.tensor_copy(
                        V[kt][:, :, 0:D_HEAD],
                        psv.rearrange("p (h d) -> p h d", h=HPG),
                    )

            def emit_scores(h, j, kt2):
                p = h // 2
                pss = ps_big.tile([128, 2 * QC], f32, tag="big", name="pss")
                off0 = 0
                for u in (0, 1):
                    kt = kt2 + u
                    delta = kt - 4 * j  # >=0 on diagonal blocks
                    off = 128 * delta if delta >= 0 else 0
                    if u == 0:
                        off0 = off
                    nc.tensor.matmul(
                        pss[:, QC * u + off : QC * (u + 1)],
                        lhsT=KT[p][:, 128 * kt : 128 * (kt + 1)],
                        rhs=QTz[h][:, QC * j + off : QC * (j + 1)],
                        start=True, stop=True,
                        skip_group_check=True,
                    )
                expt = expsb.tile([128, 2 * QC], bf16, tag="exp", name="expt")
                nc.scalar.activation(out=expt[:, off0:], in_=pss[:, off0:],
                                     func=mybir.ActivationFunctionType.Exp,
                                     scale=0.125)
                for u in (0, 1):
                    delta = kt2 + u - 4 * j
                    if delta >= 0:
                        off = 128 * delta
                        blk = slice(QC * u + off, QC * u + off + 128)
                        nc.vector.tensor_mul(expt[:, blk], expt[:, blk], mask01)
                return expt

            def emit_pv(h, j, psz, nkt, kt2, expt):
                for u in (0, 1):
                    kt = kt2 + u
                    delta = kt - 4 * j
                    off = 128 * delta if delta >= 0 else 0
                    nc.tensor.matmul(
                        psz[:, off:QC],
                        lhsT=V[kt][:, h, :],
                        rhs=expt[:, QC * u + off : QC * (u + 1)],
                        start=(kt == 0), stop=(kt == nkt - 1),
                        skip_group_check=True,
                    )

            def emit_attention(h, j):
                # k-loop with scores staggered two k-pairs ahead of PV, then
                # the per-head normalization chain (approx reciprocal -> DRAM
                # hop -> partition-broadcast DMA -> multiply) which hides
                # behind subsequent attention work
                p, r0 = h // 2, 64 * (h % 2)
                qs = slice(QC * j, QC * (j + 1))
                nkt = 4 * j + 4
                psz = ps_z.tile([D_HEAD + 1, QC], f32, tag="z", name="psz")
                pend = deque()
                for kt2 in range(0, nkt, 2):
                    expt = emit_scores(h, j, kt2)
                    pend.append((kt2, expt))
                    if len(pend) > 2:
                        kt2p, exptp = pend.popleft()
                        emit_pv(h, j, psz, nkt, kt2p, exptp)
                while pend:
                    kt2p, exptp = pend.popleft()
                    emit_pv(h, j, psz, nkt, kt2p, exptp)

                row = HPG * j + h
                dtmp = dtmpsb.tile([1, QC], f32, tag="dtmp", name="dtmp")
                nc.vector.tensor_copy(dtmp, psz[D_HEAD : D_HEAD + 1, :])
                rtmp = dtmpsb.tile([1, QC], f32, tag="rtmp", name="rtmp")
                nc.vector.reciprocal_approx_fast(rtmp, dtmp)
                nc.sync.dma_start(out=recip_d[row : row + 1, :], in_=rtmp)
                nc.vector.tensor_copy(zT[p][r0 : r0 + 64, qs], psz[0:D_HEAD, :])
                sl = recip_d[row : row + 1, :]
                rb = rbsb.tile([128, QC], f32, tag="rb", name="rb")
                nc.sync.dma_start(
                    out=rb[r0 : r0 + 64, :],
                    in_=bass.AP(tensor=sl.tensor, offset=sl.offset,
                                ap=[[0, D_HEAD]] + list(sl.ap[-1:])))
                nc.vector.tensor_mul(zT[p][r0 : r0 + 64, qs],
                                     zT[p][r0 : r0 + 64, qs],
                                     rb[r0 : r0 + 64, :])

            def emit_outproj(j):
                for c in range(4 * j, 4 * (j + 1)):
                    cs = slice(128 * c, 128 * (c + 1))
                    pso = ps_big.tile([128, D_MODEL], f32, tag="big", name="pso")
                    for p in range(NPAIR):
                        nc.tensor.matmul(pso[:, 0:512], lhsT=zT[p][:, cs],
                                         rhs=wo[p][:, 0:512],
                                         start=(p == 0), stop=(p == NPAIR - 1))
                        nc.tensor.matmul(pso[:, 512:768], lhsT=zT[p][:, cs],
                                         rhs=wo[p][:, 512:768],
                                         start=(p == 0), stop=(p == NPAIR - 1))
                    outt = outsb.tile([128, D_MODEL], f32, tag="out", name="outt")
                    nc.vector.tensor_copy(outt, pso)
                    nc.sync.dma_start(out=out_d[cs, :], in_=outt)

            # ---- schedule: j=0 attention interleaves into the projection
            # phase (PE issues in order; exp latency of the small j=0 blocks
            # hides inside projection matmul streams) ----
            emit_qkv_pair(0)
            emit_v([0, 1, 2, 3])
            emit_attention(0, 0)
            emit_attention(1, 0)
            emit_qkv_pair(1)
            emit_v([4, 5, 6, 7])
            emit_attention(2, 0)
            emit_attention(3, 0)
            emit_qkv_pair(2)
            emit_v([8, 9, 10, 11])
            emit_attention(4, 0)
            emit_attention(5, 0)
            emit_v([12, 13, 14, 15])
            for j in range(1, QC_TILES):
                for h in range(HPG):
                    emit_attention(h, j)
                    if h == 1:
                        emit_outproj(j - 1)
            emit_outproj(QC_TILES - 1)

    nc.finalize()
    return nc


def kernel(**inputs):
    x = inputs["normalized_resid_pre"]
    W_Q, W_K, W_V, W_O = inputs["W_Q"], inputs["W_K"], inputs["W_V"], inputs["W_O"]
    b_Q, b_K, b_V, b_O = inputs["b_Q"], inputs["b_K"], inputs["b_V"], inputs["b_O"]

    expected = (
        x.shape == (BATCH, SEQ, D_MODEL)
        and W_Q.shape == (N_HEADS, D_MODEL, D_HEAD)
        and W_K.shape == (N_HEADS, D_MODEL, D_HEAD)
        and W_V.shape == (N_HEADS, D_MODEL, D_HEAD)
        and W_O.shape == (N_HEADS, D_HEAD, D_MODEL)
        and not np.any(b_Q)
    )
    if not expected:
        return _numpy_ref(**inputs)

    from concourse.bass_utils import run_bass_kernel_spmd

    if "nc" not in _prog_cache:
        _prog_cache["nc"] = _build_program()
    nc = _prog_cache["nc"]

    # host-side prep: transpose + cast + pack per head-group
    xT = np.ascontiguousarray(x.transpose(0, 2, 1)).astype(BF16)  # [B, 768, 2048]
    # b_K shifts every score in a softmax row equally -> cancels exactly.
    groups = []
    for g in range(2):
        hs = slice(HPG * g, HPG * (g + 1))
        groups.append({
            "wq": np.ascontiguousarray(W_Q[hs].transpose(1, 0, 2).reshape(D_MODEL, HPG * D_HEAD)).astype(BF16),
            "wk": np.ascontiguousarray(W_K[hs].transpose(1, 0, 2).reshape(D_MODEL, HPG * D_HEAD)).astype(BF16),
            "wv": np.ascontiguousarray(W_V[hs].transpose(1, 0, 2).reshape(D_MODEL, HPG * D_HEAD)).astype(BF16),
            "wo": np.ascontiguousarray(W_O[hs].reshape(HPG * D_HEAD, D_MODEL)).astype(BF16),
        })
    ii, jj = np.arange(128)[:, None], np.arange(128)[None, :]
    mask = np.where(jj >= ii, np.float32(1.0), np.float32(0.0)).astype(BF16)

    in_maps = []
    for c in range(NCORES):
        b, g = c // 2, c % 2
        m = {"xT": xT[b], "mask": mask}
        m.update(groups[g])
        in_maps.append(m)

    trace = bool(os.environ.get("ATTN_KERNEL_TRACE"))
    res = run_bass_kernel_spmd(nc, in_maps, list(range(NCORES)), trace=trace)
    _prog_cache["last_exec_time_ns"] = res.exec_time_ns
    _prog_cache["last_results"] = res

    # b_V/b_O fold into a constant row (softmax weights sum to 1).
    const_row = np.einsum("hd,hdm->m", b_V.astype(np.float64), W_O.astype(np.float64))
    const_row = (const_row + b_O.astype(np.float64)).astype(np.float32)

    out = np.empty((BATCH, SEQ, D_MODEL), dtype=np.float32)
    for b in range(BATCH):
        out[b] = res.results[2 * b]["out"] + res.results[2 * b + 1]["out"] + const_row
    return out
